# revision 9
# baseline (speedup 1.0000x reference)
"""Trainium2 Bass kernel for a transformer decoder layer (self-attn + cross-attn + FFN).

Sharding: 8 cores = 4 batches x 2 ways. Core c handles batch c//2 and the
interleaved query row-tiles {p, p+2, p+4, p+6} (p = c%2) of that batch, which
balances the causal-attention load between the two cores of a batch. K/V
projections are computed fully per core (no collectives). The host permutes
each core's token order to [my 512 tokens, partner's 512 tokens] so the
program is SPMD-uniform; causal masks arrive as input data.

Layout: activations are kept feature-major (x^T: [d_model, tokens]) on chip so
every projection runs with a weight tile stationary and N=512 moving; all
matmuls are fp32r (tf32-like). Attention computes scores^T = K^T-tile x Q^T
directly (softmax along the partition axis), with the softmax denominator
obtained from a ones-column folded into the V matmul.
"""
import math
from contextlib import ExitStack

import numpy as np

import concourse.bass as bass
import concourse.mybir as mybir
import concourse.tile as tile
from concourse import bacc
from concourse.masks import make_identity

F32 = mybir.dt.float32
F32R = mybir.dt.float32r
AF = mybir.ActivationFunctionType
ALU = mybir.AluOpType

P = 128
D = 1024          # d_model
S = 1024          # sequence length
H = 16            # heads
DK = 64           # head dim
FF = 4096         # d_ff
NF = D // P       # 8 feature tiles
NT = S // P       # 8 token tiles
TQ = 512          # tokens per core
QS = TQ // P      # 4 query slots per core
EPS = 1e-5
NEG = -1e9
N_CORES = 8


def _s0(j, causal):
    """First participating query-slot for local k-tile j (suffix skipping).

    Local key order is [my tiles, partner tiles] (see _perm_for), so the
    causal suffix restarts at j=4. Capped at 2 to keep matmul N >= 256
    (full-rate fp32r).
    """
    if not causal:
        return 0
    return min(j % 4, 2)


def build_program(causal=True, ca_mask=False):
    nc = bacc.Bacc("TRN2", target_bir_lowering=False, debug=False,
                   num_devices=N_CORES)

    # ---------------- DRAM I/O ----------------
    x_d = nc.dram_tensor("x", [S, D], F32, kind="ExternalInput")
    enc_d = nc.dram_tensor("enc", [S, D], F32, kind="ExternalInput")
    w_d, b_d = {}, {}
    for nm in ["sa_q", "sa_k", "sa_v", "sa_o", "ca_q", "ca_k", "ca_v", "ca_o"]:
        w_d[nm] = nc.dram_tensor(nm + "_W", [D, D], F32R, kind="ExternalInput")
        b_d[nm] = nc.dram_tensor(nm + "_b", [D], F32, kind="ExternalInput")
    w_d["fc1"] = nc.dram_tensor("fc1_W", [D, FF], F32R, kind="ExternalInput")
    b_d["fc1"] = nc.dram_tensor("fc1_b", [FF], F32, kind="ExternalInput")
    w_d["fc2"] = nc.dram_tensor("fc2_W", [FF, D], F32R, kind="ExternalInput")
    b_d["fc2"] = nc.dram_tensor("fc2_b", [D], F32, kind="ExternalInput")
    ln_d = {}
    for i in (1, 2, 3):
        ln_d[f"g{i}"] = nc.dram_tensor(f"ln{i}_g", [D], F32, kind="ExternalInput")
        ln_d[f"b{i}"] = nc.dram_tensor(f"ln{i}_b", [D], F32, kind="ExternalInput")
    cmask_d = nc.dram_tensor("cmask", [NT, P, TQ], F32, kind="ExternalInput")
    camask_d = None
    if ca_mask:
        camask_d = nc.dram_tensor("camask", [NT, P, TQ], F32, kind="ExternalInput")
    y_d = nc.dram_tensor("y", [TQ, D], F32, kind="ExternalOutput")

    with tile.TileContext(nc) as tc, ExitStack() as top:
        # ---------------- constant / persistent pools ----------------
        const = top.enter_context(tc.tile_pool(name="const", bufs=1))
        identity = const.tile([P, P], F32, tag="ident")
        make_identity(nc, identity[:])
        ones_f = const.tile([P, 8], F32, tag="ones_f")
        nc.gpsimd.memset(ones_f[:], 1.0)
        ones_r = const.tile([P, 1], F32R, tag="ones")
        nc.vector.tensor_copy(ones_r[:], ones_f[:, 0:1])
        eps_t = const.tile([1, 1], F32, tag="eps")
        nc.gpsimd.memset(eps_t[:], EPS)

        def load_feat_vec(name, dram, n):
            """[n*128] dram vector -> [128, n] sbuf (partition = within-tile)."""
            t = const.tile([P, n], F32, tag=name)
            nc.sync.dma_start(t[:], dram.ap().rearrange("(o p) -> p o", p=P))
            return t

        def load_head_vec(name, dram):
            """[1024] dram vector -> [64, 16] sbuf (partition = within-head)."""
            t = const.tile([DK, H], F32, tag=name)
            nc.sync.dma_start(t[:], dram.ap().rearrange("(h p) -> p h", p=DK))
            return t

        bias_t = {nm: load_feat_vec("b_" + nm, b_d[nm], NF)
                  for nm in ["sa_q", "sa_k", "sa_o", "ca_q", "ca_k", "ca_o", "fc2"]}
        bias_t["fc1"] = load_feat_vec("b_fc1", b_d["fc1"], FF // P)
        bv_t = {"sa": load_head_vec("bv_sa", b_d["sa_v"]),
                "ca": load_head_vec("bv_ca", b_d["ca_v"])}
        ln_t = {k: load_feat_vec("ln_" + k, ln_d[k], NF) for k in ln_d}

        # psum pools: 8 banks total (2 per pool, one tag each)
        proj_ps = top.enter_context(tc.tile_pool(name="proj_ps", bufs=2, space="PSUM"))
        score_ps = top.enter_context(tc.tile_pool(name="score_ps", bufs=2, space="PSUM"))
        attn_ps = top.enter_context(tc.tile_pool(name="attn_ps", bufs=2, space="PSUM"))
        misc_ps = top.enter_context(tc.tile_pool(name="misc_ps", bufs=2, space="PSUM"))

        # small working pools reused across phases
        ln_pool = None  # opened after self-attention (first LN call)
        st_pool = top.enter_context(tc.tile_pool(name="st_pool", bufs=2))

        # residual-sum tiles: reused (ring) for sa-res, ca-res, ffn-res
        res_pool = top.enter_context(tc.tile_pool(name="res_pool", bufs=NF))
        # x1T -> x2T -> yT reuse the same ring
        xn_pool = top.enter_context(tc.tile_pool(name="xn_pool", bufs=NF))

        # ---------------- helpers ----------------
        def load_transposed(dram, dst_tiles, pool, nt=NT):
            """Token-major [S, D] dram -> feature-major fp32r tiles [128, nt*128]."""
            for f in range(NF):
                for t in range(nt):
                    tm = pool.tile([P, P], F32, tag="tm", bufs=3)
                    nc.sync.dma_start(
                        tm[:], dram[t * P:(t + 1) * P, f * P:(f + 1) * P])
                    ps = misc_ps.tile([P, TQ], F32, tag="misc")
                    nc.tensor.transpose(ps[:, 0:P], tm[:], identity[:])
                    nc.vector.tensor_copy(dst_tiles[f][:, t * P:(t + 1) * P],
                                          ps[:, 0:P])

        def dense_cols(Wd, nf, nog, ncols, rhs_tiles, wpool, wtag, wbufs, evict):
            """Feature-major dense layer: out^T[og] = sum_f W[f,og]^T @ rhs[f]."""
            for og in range(nog):
                wt = wpool.tile([P, nf * P], F32R, tag=wtag, bufs=wbufs)
                nc.sync.dma_start(
                    wt[:].rearrange("p (f o) -> p f o", o=P),
                    Wd[:, og * P:(og + 1) * P].rearrange("(f p) o -> p f o", p=P))
                for cc in range(math.ceil(ncols / 512)):
                    c0, c1 = cc * 512, min(ncols, cc * 512 + 512)
                    ps = proj_ps.tile([P, c1 - c0], F32, tag="proj")
                    for f in range(nf):
                        nc.tensor.matmul(ps[:], wt[:, f * P:(f + 1) * P],
                                         rhs_tiles[f][:, c0:c1],
                                         start=(f == 0), stop=(f == nf - 1))
                    evict(og, cc, c0, c1, ps)

        def layer_norm(src_tiles, dst_tiles, g_t, b_t, dst_dtype=F32R):
            """Feature-major LN over the partition (feature) axis; 512 tokens."""
            sx_ps = misc_ps.tile([1, TQ], F32, tag="misc")
            sq_ps = misc_ps.tile([1, TQ], F32, tag="misc")
            for og in range(NF):
                nc.tensor.matmul(sx_ps[:], ones_r[:], src_tiles[og][:, 0:TQ],
                                 start=(og == 0), stop=(og == NF - 1))
            for og in range(NF):
                sq = ln_pool.tile([P, TQ], F32R, tag="sq", bufs=1)
                nc.scalar.activation(sq[:], src_tiles[og][:, 0:TQ], AF.Square)
                nc.tensor.matmul(sq_ps[:], ones_r[:], sq[:],
                                 start=(og == 0), stop=(og == NF - 1))
            mu = st_pool.tile([1, TQ], F32, tag="s1", bufs=1)
            nc.scalar.mul(mu[:], sx_ps[:], 1.0 / D)
            msq = st_pool.tile([1, TQ], F32, tag="s2", bufs=1)
            nc.scalar.mul(msq[:], sq_ps[:], 1.0 / D)
            mu2 = st_pool.tile([1, TQ], F32, tag="s3", bufs=1)
            nc.vector.tensor_tensor(mu2[:], mu[:], mu[:], ALU.mult)
            var = st_pool.tile([1, TQ], F32, tag="s4", bufs=1)
            nc.vector.tensor_tensor(var[:], msq[:], mu2[:], ALU.subtract)
            sd = st_pool.tile([1, TQ], F32, tag="s5", bufs=1)
            nc.scalar.activation(sd[:], var[:], AF.Sqrt, bias=eps_t[:])
            rstd = st_pool.tile([1, TQ], F32, tag="s6", bufs=1)
            nc.vector.reciprocal(rstd[:], sd[:])
            mA = st_pool.tile([1, TQ], F32, tag="s7", bufs=1)
            nc.vector.tensor_tensor(mA[:], mu[:], rstd[:], ALU.mult)
            nB = st_pool.tile([1, TQ], F32, tag="s8", bufs=1)
            nc.scalar.mul(nB[:], mA[:], -1.0)
            Ab = ln_pool.tile([P, TQ], F32, tag="Ab", bufs=1)
            nc.gpsimd.partition_broadcast(Ab[:], rstd[:])
            Bb = ln_pool.tile([P, TQ], F32, tag="Bb", bufs=1)
            nc.gpsimd.partition_broadcast(Bb[:], nB[:])
            for og in range(NF):
                t1 = ln_pool.tile([P, TQ], F32, tag="t1", bufs=1)
                nc.vector.tensor_tensor(t1[:], src_tiles[og][:, 0:TQ], Ab[:],
                                        ALU.mult)
                t2 = ln_pool.tile([P, TQ], F32, tag="t2", bufs=1)
                nc.vector.tensor_tensor(t2[:], t1[:], Bb[:], ALU.add)
                nc.scalar.activation(dst_tiles[og][:], t2[:], AF.Identity,
                                     bias=b_t[:, og:og + 1], scale=g_t[:, og:og + 1])

        def attention(prefix, qsrc_tiles, kvsrc_tiles, masked, mask_dram,
                      res_tiles, out_tiles, phase_stack, post_proj=None):
            """One multi-head attention block, feature-major.

            qsrc:  tiles [128, >=512] fp32r (queries = cols 0:512)
            kvsrc: tiles [128, 1024] fp32r (key/value source tokens)
            res:   residual tiles [128, >=512] (added after O-proj)
            out:   result tiles [128, 512]
            """
            kv_pool = phase_stack.enter_context(
                tc.tile_pool(name=prefix + "_kv", bufs=1))
            KT = [kv_pool.tile([P, S], F32R, tag="KT", bufs=NF, name=f"KT{i}")
                  for i in range(NF)]
            QT = [kv_pool.tile([P, TQ], F32R, tag="QT", bufs=NF, name=f"QT{i}")
                  for i in range(NF)]
            vaug = [[kv_pool.tile([P, 8 * 65], F32R, tag="vaug", bufs=2 * NT,
                                  name=f"vaug{t}_{o}")
                     for o in range(2)] for t in range(NT)]
            wpool = phase_stack.enter_context(
                tc.tile_pool(name=prefix + "_w", bufs=1))

            # K projection (with bias), full kv tokens
            def k_evict(og, cc, c0, c1, ps):
                nc.scalar.activation(KT[og][:, c0:c1], ps[:], AF.Identity,
                                     bias=bias_t[prefix + "_k"][:, og:og + 1])
            dense_cols(w_d[prefix + "_k"], NF, NF, S, kvsrc_tiles,
                       wpool, "w", 2, k_evict)

            # Q projection (with bias), my 512 tokens
            def q_evict(og, cc, c0, c1, ps):
                nc.scalar.activation(QT[og][:, c0:c1], ps[:], AF.Identity,
                                     bias=bias_t[prefix + "_q"][:, og:og + 1])
            dense_cols(w_d[prefix + "_q"], NF, NF, TQ, qsrc_tiles,
                       wpool, "w", 2, q_evict)

            # V projection, token-major, ones column appended per head.
            # Stream W_v row-panels in 512-column halves to bound SBUF.
            with tc.tile_pool(name=prefix + "_wv", bufs=1) as wrow:
                for oh in range(2):
                    wvh = []
                    for f in range(NF):
                        wv = wrow.tile([P, 512], F32R, tag="wv", bufs=NF)
                        nc.sync.dma_start(
                            wv[:],
                            w_d[prefix + "_v"][f * P:(f + 1) * P,
                                               oh * 512:(oh + 1) * 512])
                        wvh.append(wv)
                    for tt in range(NT):
                        ps = proj_ps.tile([P, 512], F32, tag="proj")
                        for f in range(NF):
                            nc.tensor.matmul(
                                ps[:], kvsrc_tiles[f][:, tt * P:(tt + 1) * P],
                                wvh[f][:], start=(f == 0), stop=(f == NF - 1))
                        vt = vaug[tt][oh]
                        v3 = vt[:].rearrange("p (h e) -> p h e", e=65)
                        nc.vector.tensor_copy(
                            v3[:, :, 0:DK],
                            ps[:].rearrange("p (h e) -> p h e", e=DK))
                        nc.vector.tensor_copy(
                            v3[:, :, DK:DK + 1],
                            ones_f[:].rearrange("p (h o) -> p h o", o=1))

            if post_proj is not None:
                post_proj()

            # attention core, one head pair at a time
            core_pool = phase_stack.enter_context(
                tc.tile_pool(name=prefix + "_core", bufs=1))
            pair_tiles = [core_pool.tile([P, TQ], F32R, tag="pair", bufs=NF,
                                         name=f"pair{i}")
                          for i in range(NF)]
            for hp in range(NF):
                ap_h = [attn_ps.tile([65, TQ], F32, tag="attn", name=f"apsum{hp}_{i}")
                        for i in range(2)]
                for j in range(NT):
                    c0 = _s0(j, masked and causal) * P
                    if masked:
                        cmt = core_pool.tile([P, TQ], F32, tag="cm", bufs=3)
                        nc.sync.dma_start(cmt[:, c0:TQ], mask_dram[j][:, c0:TQ])
                    for idx in range(2):
                        h = 2 * hp + idx
                        base = idx * DK
                        sps = score_ps.tile([P, TQ], F32, tag="score")
                        nc.tensor.matmul(
                            sps[:, c0:TQ],
                            KT[hp][base:base + DK, j * P:(j + 1) * P],
                            QT[hp][base:base + DK, c0:TQ],
                            start=True, stop=True)
                        probs = core_pool.tile([P, TQ], F32R, tag="probs")
                        if masked:
                            ptmp = core_pool.tile([P, TQ], F32, tag="ptmp")
                            nc.vector.scalar_tensor_tensor(
                                ptmp[:, c0:TQ], sps[:, c0:TQ], 0.125,
                                cmt[:, c0:TQ], ALU.mult, ALU.add)
                            nc.scalar.activation(probs[:, c0:TQ],
                                                 ptmp[:, c0:TQ], AF.Exp)
                        else:
                            nc.scalar.activation(probs[:, c0:TQ], sps[:, c0:TQ],
                                                 AF.Exp, scale=0.125)
                        nc.tensor.matmul(
                            ap_h[idx][0:65, c0:TQ],
                            vaug[j][h // 8][:, (h % 8) * 65:(h % 8) * 65 + 65],
                            probs[:, c0:TQ],
                            start=(j == 0), stop=(j == NT - 1))
                for idx in range(2):
                    h = 2 * hp + idx
                    r = st_pool.tile([1, TQ], F32, tag="rcp")
                    nc.vector.reciprocal(r[:], ap_h[idx][DK:DK + 1, :])
                    rb = core_pool.tile([DK, TQ], F32, tag="rb")
                    nc.gpsimd.partition_broadcast(rb[:], r[:])
                    tmp = core_pool.tile([DK, TQ], F32, tag="atmp")
                    nc.vector.tensor_tensor(tmp[:], ap_h[idx][0:DK, :], rb[:],
                                            ALU.mult)
                    hfin = core_pool.tile([DK, TQ], F32R, tag="hfin")
                    nc.scalar.activation(hfin[:], tmp[:], AF.Identity,
                                         bias=bv_t[prefix][:, h:h + 1])
                    nc.sync.dma_start(
                        pair_tiles[hp][idx * DK:(idx + 1) * DK, :], hfin[:])

            # O projection + bias + residual
            def o_evict(og, cc, c0, c1, ps):
                nc.vector.scalar_tensor_tensor(
                    out_tiles[og][:], ps[:], bias_t[prefix + "_o"][:, og:og + 1],
                    res_tiles[og][:, 0:TQ], ALU.add, ALU.add)
            dense_cols(w_d[prefix + "_o"], NF, NF, TQ, pair_tiles,
                       wpool, "w", 2, o_evict)

        # ================= self attention =================
        sa_st = ExitStack()
        xTr_st = ExitStack()
        xTr_pool = xTr_st.enter_context(
            tc.tile_pool(name="xTr", bufs=1, side="right"))
        xTr = [xTr_pool.tile([P, S], F32R, tag="xTr", bufs=NF, name=f"xTr{i}")
               for i in range(NF)]
        with tc.tile_pool(name="tm_x", bufs=1) as tm_pool:
            load_transposed(x_d, xTr, tm_pool)

        xq_pool = sa_st.enter_context(tc.tile_pool(name="xTq", bufs=1))
        xTq = []

        def sa_post_proj():
            for f in range(NF):
                xt = xq_pool.tile([P, TQ], F32R, tag="xTq", bufs=NF,
                                  name=f"xTq{f}")
                nc.vector.tensor_copy(xt[:], xTr[f][:, 0:TQ])
                xTq.append(xt)
            xTr_st.close()

        saresT = [res_pool.tile([P, TQ], F32R, tag="res", bufs=NF,
                                  name=f"sares{i}")
                  for i in range(NF)]
        attention("sa", xTr, xTr, True, cmask_d, xTq, saresT, sa_st,
                  post_proj=sa_post_proj)
        sa_st.close()

        # ================= LN1 =================
        ln_pool = top.enter_context(tc.tile_pool(name="ln_pool", bufs=1))
        x1T = [xn_pool.tile([P, TQ], F32R, tag="xn", bufs=NF, name=f"x1T{i}")
               for i in range(NF)]
        layer_norm(saresT, x1T, ln_t["g1"], ln_t["b1"])

        # ================= cross attention =================
        ca_st = ExitStack()
        encT_st = ExitStack()
        encT_pool = encT_st.enter_context(
            tc.tile_pool(name="encT", bufs=1, side="right"))
        encT = [encT_pool.tile([P, S], F32R, tag="encT", bufs=NF,
                               name=f"encT{i}")
                for i in range(NF)]
        with tc.tile_pool(name="tm_e", bufs=1) as tm_pool:
            load_transposed(enc_d, encT, tm_pool)

        caresT = [res_pool.tile([P, TQ], F32R, tag="res", bufs=NF,
                                  name=f"cares{i}")
                  for i in range(NF)]
        attention("ca", x1T, encT, ca_mask, camask_d, x1T, caresT, ca_st,
                  post_proj=encT_st.close)
        ca_st.close()

        x2T = [xn_pool.tile([P, TQ], F32R, tag="xn", bufs=NF, name=f"x2T{i}")
               for i in range(NF)]
        layer_norm(caresT, x2T, ln_t["g2"], ln_t["b2"])

        # ================= FFN =================
        ffn_st = ExitStack()
        h1_pool = ffn_st.enter_context(tc.tile_pool(name="h1T", bufs=1))
        h1T = [h1_pool.tile([P, TQ], F32R, tag="h1T", bufs=FF // P,
                            name=f"h1T{i}")
               for i in range(FF // P)]
        wf_pool = ffn_st.enter_context(tc.tile_pool(name="wf", bufs=1))

        def fc1_evict(og, cc, c0, c1, ps):
            nc.scalar.activation(h1T[og][:], ps[:], AF.Relu,
                                 bias=bias_t["fc1"][:, og:og + 1])
        dense_cols(w_d["fc1"], NF, FF // P, TQ, x2T, wf_pool, "w1", 3, fc1_evict)

        ffresT = [res_pool.tile([P, TQ], F32R, tag="res", bufs=NF,
                                  name=f"ffres{i}")
                  for i in range(NF)]

        def fc2_evict(og, cc, c0, c1, ps):
            nc.vector.scalar_tensor_tensor(
                ffresT[og][:], ps[:], bias_t["fc2"][:, og:og + 1],
                x2T[og][:, 0:TQ], ALU.add, ALU.add)
        dense_cols(w_d["fc2"], FF // P, NF, TQ, h1T, wf_pool, "w2", 2, fc2_evict)
        ffn_st.close()

        yT = [xn_pool.tile([P, TQ], F32, tag="xn", bufs=NF, name=f"yT{i}")
              for i in range(NF)]
        layer_norm(ffresT, yT, ln_t["g3"], ln_t["b3"], dst_dtype=F32)

        # ================= transpose out + store =================
        with tc.tile_pool(name="out", bufs=1) as out_pool:
            out_sb = [out_pool.tile([P, D], F32, tag="out", bufs=QS,
                                name=f"osb{i}")
                      for i in range(QS)]
            for og in range(NF):
                for t in range(QS):
                    ps = misc_ps.tile([P, TQ], F32, tag="misc")
                    nc.tensor.transpose(ps[:, 0:P], yT[og][:, t * P:(t + 1) * P],
                                        identity[:])
                    nc.vector.tensor_copy(out_sb[t][:, og * P:(og + 1) * P],
                                          ps[:, 0:P])
            for t in range(QS):
                nc.sync.dma_start(y_d[t * P:(t + 1) * P, :], out_sb[t][:])

    nc.compile()
    return nc


# =====================================================================
# Host side
# =====================================================================

def _perm_for(p):
    mine = np.concatenate(
        [np.arange(t * P, (t + 1) * P) for t in range(p, NT, 2)])
    rest = np.concatenate(
        [np.arange(t * P, (t + 1) * P) for t in range(1 - p, NT, 2)])
    return np.concatenate([mine, rest])


def _build_masks(tgt, perm, causal):
    """Additive masks [NT, P, TQ] in permuted token order for one parity."""
    q_glob = perm[:TQ]
    m = tgt[np.ix_(q_glob, perm)]                  # [TQ, S], 1 = keep
    add = np.where(m.T == 1, 0.0, NEG).astype(np.float32)   # [S, TQ]
    out = np.ascontiguousarray(add.reshape(NT, P, TQ))
    if causal:
        # verify suffix skipping is sound: slots below s0(j) fully masked
        for j in range(NT):
            c0 = _s0(j, True) * P
            if c0 > 0 and not (out[j, :, :c0] == NEG).all():
                return None
    return out


_CACHE = {}


def _get_runner(causal, ca_mask):
    key = (causal, ca_mask)
    if key in _CACHE:
        return _CACHE[key]
    import jax
    from jax.sharding import Mesh, PartitionSpec
    from jax.experimental.shard_map import shard_map
    from concourse import bass2jax

    nc = build_program(causal=causal, ca_mask=ca_mask)
    bass2jax.install_neuronx_cc_hook()

    pid_name = (nc.partition_id_tensor.name
                if nc.partition_id_tensor is not None else None)
    in_names, out_names, out_avals, zero_outs = [], [], [], []
    for alloc in nc.m.functions[0].allocations:
        if not isinstance(alloc, mybir.MemoryLocationSet):
            continue
        name = alloc.memorylocations[0].name
        if alloc.kind == "ExternalInput":
            if name != pid_name:
                in_names.append(name)
        elif alloc.kind == "ExternalOutput":
            out_names.append(name)
            shape = tuple(alloc.tensor_shape)
            dtype = mybir.dt.np(alloc.dtype)
            out_avals.append(jax.core.ShapedArray(shape, dtype))
            zero_outs.append(np.zeros(shape, dtype))
    n_params = len(in_names)
    all_in_names = in_names + out_names
    if pid_name is not None:
        all_in_names = all_in_names + [pid_name]

    def _body(*args):
        operands = list(args)
        if pid_name is not None:
            operands.append(bass2jax.partition_id_tensor())
        outs = bass2jax._bass_exec_p.bind(
            *operands,
            out_avals=tuple(out_avals),
            in_names=tuple(all_in_names),
            out_names=tuple(out_names),
            lowering_input_output_aliases=(),
            sim_require_finite=True,
            sim_require_nnan=True,
            nc=nc,
        )
        return tuple(outs)

    devices = jax.devices()[:N_CORES]
    mesh = Mesh(np.asarray(devices), ("core",))
    n_outs = len(out_names)
    sharded = jax.jit(
        shard_map(_body, mesh=mesh,
                  in_specs=(PartitionSpec("core"),) * (n_params + n_outs),
                  out_specs=(PartitionSpec("core"),) * n_outs,
                  check_rep=False),
        donate_argnums=tuple(range(n_params, n_params + n_outs)),
        keep_unused=True,
    )

    runner = (sharded, in_names, out_names, zero_outs)
    _CACHE[key] = runner
    return runner


def _prepare_inputs(x, enc_output, src_mask, tgt_mask, params):
    x = np.asarray(x, np.float32)
    enc = np.asarray(enc_output, np.float32)
    tgt = np.asarray(tgt_mask).reshape(S, S)
    src = np.asarray(src_mask).reshape(-1)
    ca_mask = not (src == 1).all()

    perms = [_perm_for(0), _perm_for(1)]
    causal = True
    masks = [_build_masks(tgt, perms[p], True) for p in (0, 1)]
    if masks[0] is None or masks[1] is None:
        causal = False
        masks = [_build_masks(tgt, perms[p], False) for p in (0, 1)]

    in_maps = []
    for c in range(N_CORES):
        b, p = c // 2, c % 2
        m = {
            "x": np.ascontiguousarray(x[b][perms[p]]),
            "enc": np.ascontiguousarray(enc[b]),
            "cmask": masks[p],
        }
        for nm in ["sa_q", "sa_k", "sa_v", "sa_o", "ca_q", "ca_k", "ca_v", "ca_o"]:
            m[nm + "_W"] = np.asarray(params[nm + "_W"], np.float32)
            m[nm + "_b"] = np.asarray(params[nm + "_b"], np.float32)
        m["fc1_W"] = np.asarray(params["fc1_W"], np.float32)
        m["fc1_b"] = np.asarray(params["fc1_b"], np.float32)
        m["fc2_W"] = np.asarray(params["fc2_W"], np.float32)
        m["fc2_b"] = np.asarray(params["fc2_b"], np.float32)
        for i in (1, 2, 3):
            m[f"ln{i}_g"] = np.asarray(params[f"ln{i}_g"], np.float32)
            m[f"ln{i}_b"] = np.asarray(params[f"ln{i}_b"], np.float32)
        if ca_mask:
            sm = np.where(src == 1, 0.0, NEG).astype(np.float32)
            cam = np.broadcast_to(sm[:, None], (S, TQ)).reshape(NT, P, TQ)
            m["camask"] = np.ascontiguousarray(cam)
        in_maps.append(m)
    return in_maps, causal, ca_mask, perms


def kernel(x, enc_output, src_mask, tgt_mask, params):
    in_maps, causal, ca_mask, perms = _prepare_inputs(
        x, enc_output, src_mask, tgt_mask, params)
    sharded, in_names, out_names, zero_outs = _get_runner(causal, ca_mask)

    concat_in = [np.concatenate([m[nm] for m in in_maps], axis=0)
                 for nm in in_names]
    concat_zeros = [np.zeros((N_CORES * z.shape[0], *z.shape[1:]), z.dtype)
                    for z in zero_outs]
    out_arrs = sharded(*concat_in, *concat_zeros)
    yi = out_names.index("y")
    ys = np.asarray(out_arrs[yi]).reshape(N_CORES, TQ, D)

    out = np.empty((N_CORES // 2, S, D), np.float32)
    for c in range(N_CORES):
        b, p = c // 2, c % 2
        out[b][perms[p][:TQ]] = ys[c]
    return out


# revision 16
# speedup vs baseline: 2227.6878x; 2227.6878x over previous
"""Trainium2 Bass kernel for a transformer decoder layer (self-attn + cross-attn + FFN).

Sharding: 8 cores = 4 batches x 2 ways. Core c handles batch c//2 and the
interleaved query row-tiles {p, p+2, p+4, p+6} (p = c%2) of that batch, which
balances the causal-attention load between the two cores of a batch. K/V
projections are computed fully per core (no collectives). The host permutes
each core's token order to [my 512 tokens, partner's 512 tokens] so the
program is SPMD-uniform; causal masks arrive as input data.

Layout: activations are kept feature-major (x^T: [d_model, tokens]) on chip so
every projection runs with a weight tile stationary and N=512 moving; all
matmuls are fp32r (tf32-like). Attention computes scores^T = K^T-tile x Q^T
directly (softmax along the partition axis), with the softmax denominator
obtained from a ones-column folded into the V matmul.
"""
import math
from contextlib import ExitStack

import numpy as np

import concourse.bass as bass
import concourse.mybir as mybir
import concourse.tile as tile
from concourse import bacc
from concourse.masks import make_identity

F32 = mybir.dt.float32
F32R = mybir.dt.float32r
AF = mybir.ActivationFunctionType
ALU = mybir.AluOpType

P = 128
D = 1024          # d_model
S = 1024          # sequence length
H = 16            # heads
DK = 64           # head dim
FF = 4096         # d_ff
NF = D // P       # 8 feature tiles
NT = S // P       # 8 token tiles
TQ = 512          # tokens per core
QS = TQ // P      # 4 query slots per core
EPS = 1e-5
NEG = -1e9
N_CORES = 8

# batched constant-vector layout: (name, column offset, n feature tiles)
CVEC_ORDER = [
    ("sa_q", 0, 8), ("sa_k", 8, 8), ("sa_o", 16, 8),
    ("ca_q", 24, 8), ("ca_k", 32, 8), ("ca_o", 40, 8),
    ("fc2", 48, 8), ("fc1", 56, 32),
    ("g1", 88, 8), ("b1", 96, 8), ("g2", 104, 8), ("b2", 112, 8),
    ("g3", 120, 8), ("b3", 128, 8),
]
CVEC_COLS = 136


def _s0(j, causal):
    """First participating query-slot for local k-tile j (suffix skipping).

    Local key order is [my tiles, partner tiles] (see _perm_for), so the
    causal suffix restarts at j=4. Capped at 2 to keep matmul N >= 256
    (full-rate fp32r).
    """
    if not causal:
        return 0
    return min(j % 4, 2)


def build_program(causal=True, ca_mask=False):
    nc = bacc.Bacc("TRN2", target_bir_lowering=False, debug=False,
                   num_devices=N_CORES)

    # ---------------- DRAM I/O ----------------
    x_d = nc.dram_tensor("x", [S, D], F32, kind="ExternalInput")
    enc_d = nc.dram_tensor("enc", [S, D], F32, kind="ExternalInput")
    w_d = {}
    for nm in ["sa_q", "sa_k", "sa_v", "sa_o", "ca_q", "ca_k", "ca_v", "ca_o"]:
        w_d[nm] = nc.dram_tensor(nm + "_W", [D, D], F32R, kind="ExternalInput")
    w_d["fc1"] = nc.dram_tensor("fc1_W", [D, FF], F32R, kind="ExternalInput")
    w_d["fc2"] = nc.dram_tensor("fc2_W", [FF, D], F32R, kind="ExternalInput")
    # all feature-tile bias/ln vectors batched into one [128, 136] tensor and
    # the per-head V biases into one [64, 32] tensor (see CVEC_ORDER)
    cvec_d = nc.dram_tensor("cvec", [P, CVEC_COLS], F32, kind="ExternalInput")
    cvec64_d = nc.dram_tensor("cvec64", [DK, 2 * H], F32, kind="ExternalInput")
    cmask_d = None
    pm_d = None
    if causal:
        # parity scalar: 0.0 (even cores) / 1.0 (odd) for partner-half blocks
        pm_d = nc.dram_tensor("pm", [P, 1], F32, kind="ExternalInput")
    else:
        cmask_d = nc.dram_tensor("cmask", [NT, P, TQ], F32, kind="ExternalInput")
    camask_d = None
    if ca_mask:
        camask_d = nc.dram_tensor("camask", [NT, P, TQ], F32, kind="ExternalInput")
    y_d = nc.dram_tensor("y", [TQ, D], F32, kind="ExternalOutput")

    with tile.TileContext(nc) as tc, ExitStack() as top:
        # ---------------- constant / persistent pools ----------------
        const = top.enter_context(tc.tile_pool(name="const", bufs=1))
        identity = const.tile([P, P], F32, tag="ident")
        make_identity(nc, identity[:])
        ones_f = const.tile([P, 8], F32, tag="ones_f")
        nc.gpsimd.memset(ones_f[:], 1.0)
        ones_r = const.tile([P, 1], F32R, tag="ones")
        nc.vector.tensor_copy(ones_r[:], ones_f[:, 0:1])
        zcol = const.tile([P, 1], F32, tag="zcol")
        nc.gpsimd.memset(zcol[:], 0.0)
        tri = None
        pm_t = None
        if causal:
            # tri[k, q] = 1 if q >= k else 0 (keep at-or-below diagonal)
            tri = const.tile([P, P], F32, tag="tri")
            nc.gpsimd.memset(tri[:], 1.0)
            nc.gpsimd.affine_select(
                out=tri[:], in_=tri[:], compare_op=ALU.is_ge, fill=0.0,
                base=0, pattern=[[1, P]], channel_multiplier=-1)
            pm_t = const.tile([P, 1], F32, tag="pm")
            nc.scalar.dma_start(pm_t[:], pm_d[:, :])
        eps_t = const.tile([1, 1], F32, tag="eps")
        nc.gpsimd.memset(eps_t[:], EPS)

        cvec_t = const.tile([P, CVEC_COLS], F32, tag="cvec")
        nc.scalar.dma_start(cvec_t[:], cvec_d[:, :])
        cvec64_t = const.tile([DK, 2 * H], F32, tag="cvec64")
        nc.scalar.dma_start(cvec64_t[:], cvec64_d[:, :])
        bias_t, ln_t = {}, {}
        for nm, off, n in CVEC_ORDER:
            ap = cvec_t[:, off:off + n]
            if nm.startswith("g") or (nm.startswith("b") and len(nm) == 2):
                ln_t[nm] = ap
            else:
                bias_t[nm] = ap
        bv_t = {"sa": cvec64_t[:, 0:H], "ca": cvec64_t[:, H:2 * H]}

        # psum pools: 8 banks total (proj shared with transposes/LN-stats)
        proj_ps = top.enter_context(tc.tile_pool(name="proj_ps", bufs=2, space="PSUM"))
        score_ps = top.enter_context(tc.tile_pool(name="score_ps", bufs=3, space="PSUM"))
        attn_ps = top.enter_context(tc.tile_pool(name="attn_ps", bufs=3, space="PSUM"))
        misc_ps = proj_ps

        # small working pools reused across phases
        ln_pool = None  # opened after self-attention (first LN call)
        st_pool = top.enter_context(tc.tile_pool(name="st_pool", bufs=2))


        # ---------------- helpers ----------------
        def load_transposed(dram, dst_tiles, pool, nt=NT):
            """Token-major [S, D] dram -> feature-major fp32r tiles [128, nt*128]."""
            for t in range(nt):
                tm = pool.tile([P, D], F32, tag="tm", bufs=2, name=f"tm{t}")
                nc.scalar.dma_start(tm[:], dram[t * P:(t + 1) * P, :])
                for f in range(NF):
                    ps = misc_ps.tile([P, TQ], F32, tag="proj")
                    nc.tensor.transpose(ps[:, 0:P],
                                        tm[:, f * P:(f + 1) * P], identity[:])
                    nc.vector.tensor_copy(dst_tiles[f][:, t * P:(t + 1) * P],
                                          ps[:, 0:P])

        def dense_cols(Wd, nf, nog, ncols, rhs_tiles, wpool, wtag, wbufs, evict,
                       ogb=2):
            """Feature-major dense layer: out^T[og] = sum_f W[f,og]^T @ rhs[f].

            Weight DMAs fetch ogb output-tiles at once (bigger descriptors,
            fewer transfers)."""
            assert nog % ogb == 0
            for og0 in range(0, nog, ogb):
                wt = wpool.tile([P, nf * ogb * P], F32R, tag=wtag, bufs=wbufs,
                                name=f"{wtag}w{og0}")
                nc.sync.dma_start(
                    wt[:].rearrange("p (f o) -> p f o", o=ogb * P),
                    Wd[:, og0 * P:(og0 + ogb) * P].rearrange(
                        "(f p) o -> p f o", p=P))
                for gi in range(ogb):
                    og = og0 + gi
                    for cc in range(math.ceil(ncols / 512)):
                        c0, c1 = cc * 512, min(ncols, cc * 512 + 512)
                        ps = proj_ps.tile([P, c1 - c0], F32, tag="proj")
                        for f in range(nf):
                            nc.tensor.matmul(
                                ps[:],
                                wt[:, (f * ogb + gi) * P:(f * ogb + gi + 1) * P],
                                rhs_tiles[f][:, c0:c1],
                                start=(f == 0), stop=(f == nf - 1))
                        evict(og, cc, c0, c1, ps)

        def layer_norm(src_tiles, dst_tiles, g_t, b_t, dst_dtype=F32R):
            """Feature-major LN over the partition (feature) axis; 512 tokens."""
            sx_ps = misc_ps.tile([1, TQ], F32, tag="proj")
            sq_ps = misc_ps.tile([1, TQ], F32, tag="proj")
            for og in range(NF):
                nc.tensor.matmul(sx_ps[:], ones_r[:], src_tiles[og][:, 0:TQ],
                                 start=(og == 0), stop=(og == NF - 1))
            for og in range(NF):
                sq = ln_pool.tile([P, TQ], F32R, tag="sq", bufs=1)
                nc.scalar.activation(sq[:], src_tiles[og][:, 0:TQ], AF.Square)
                nc.tensor.matmul(sq_ps[:], ones_r[:], sq[:],
                                 start=(og == 0), stop=(og == NF - 1))
            mu = st_pool.tile([1, TQ], F32, tag="s1", bufs=1)
            nc.scalar.mul(mu[:], sx_ps[:], 1.0 / D)
            msq = st_pool.tile([1, TQ], F32, tag="s2", bufs=1)
            nc.scalar.mul(msq[:], sq_ps[:], 1.0 / D)
            mu2 = st_pool.tile([1, TQ], F32, tag="s3", bufs=1)
            nc.vector.tensor_tensor(mu2[:], mu[:], mu[:], ALU.mult)
            var = st_pool.tile([1, TQ], F32, tag="s4", bufs=1)
            nc.vector.tensor_tensor(var[:], msq[:], mu2[:], ALU.subtract)
            sd = st_pool.tile([1, TQ], F32, tag="s5", bufs=1)
            nc.scalar.activation(sd[:], var[:], AF.Sqrt, bias=eps_t[:])
            rstd = st_pool.tile([1, TQ], F32, tag="s6", bufs=1)
            nc.vector.reciprocal(rstd[:], sd[:])
            mA = st_pool.tile([1, TQ], F32, tag="s7", bufs=1)
            nc.vector.tensor_tensor(mA[:], mu[:], rstd[:], ALU.mult)
            nB = st_pool.tile([1, TQ], F32, tag="s8", bufs=1)
            nc.scalar.mul(nB[:], mA[:], -1.0)
            Ab = ln_pool.tile([P, TQ], F32, tag="Ab", bufs=1)
            nc.gpsimd.partition_broadcast(Ab[:], rstd[:])
            Bb = ln_pool.tile([P, TQ], F32, tag="Bb", bufs=1)
            nc.gpsimd.partition_broadcast(Bb[:], nB[:])
            for og in range(NF):
                t1 = ln_pool.tile([P, TQ], F32, tag="t1", bufs=1)
                nc.vector.tensor_tensor(t1[:], src_tiles[og][:, 0:TQ], Ab[:],
                                        ALU.mult)
                t2 = ln_pool.tile([P, TQ], F32, tag="t2", bufs=1)
                nc.vector.tensor_tensor(t2[:], t1[:], Bb[:], ALU.add)
                nc.scalar.activation(dst_tiles[og][:], t2[:], AF.Identity,
                                     bias=b_t[:, og:og + 1], scale=g_t[:, og:og + 1])

        def attention(prefix, qsrc_tiles, kvsrc_tiles, mask_mode, mask_dram,
                      res_tiles, out_tiles, phase_stack, post_proj=None):
            # mask_mode: None | "causal" (tri/pm fast path) | "generic"
            masked = mask_mode is not None
            """One multi-head attention block, feature-major.

            qsrc:  tiles [128, >=512] fp32r (queries = cols 0:512)
            kvsrc: tiles [128, 1024] fp32r (key/value source tokens)
            res:   residual tiles [128, >=512] (added after O-proj)
            out:   result tiles [128, 512]
            """
            kv_pool = phase_stack.enter_context(
                tc.tile_pool(name=prefix + "_kv", bufs=1))
            KT = [kv_pool.tile([P, S], F32R, tag="KT", bufs=NF, name=f"KT{i}")
                  for i in range(NF)]
            QT = [kv_pool.tile([P, TQ], F32R, tag="QT", bufs=NF, name=f"QT{i}")
                  for i in range(NF)]
            vaug = [[kv_pool.tile([P, 8 * 65], F32R, tag="vaug", bufs=2 * NT,
                                  name=f"vaug{t}_{o}")
                     for o in range(2)] for t in range(NT)]
            wpool = phase_stack.enter_context(
                tc.tile_pool(name=prefix + "_w", bufs=1))

            # K projection (with bias), full kv tokens
            def k_evict(og, cc, c0, c1, ps):
                nc.scalar.activation(KT[og][:, c0:c1], ps[:], AF.Identity,
                                     bias=bias_t[prefix + "_k"][:, og:og + 1])
            dense_cols(w_d[prefix + "_k"], NF, NF, S, kvsrc_tiles,
                       wpool, "w", 2, k_evict)

            # Q projection (with bias), my 512 tokens
            def q_evict(og, cc, c0, c1, ps):
                nc.scalar.activation(QT[og][:, c0:c1], ps[:], AF.Identity,
                                     bias=bias_t[prefix + "_q"][:, og:og + 1])
            dense_cols(w_d[prefix + "_q"], NF, NF, TQ, qsrc_tiles,
                       wpool, "w", 2, q_evict)

            # V projection, token-major, ones column appended per head.
            # Stream W_v row-panels in 512-column halves to bound SBUF.
            with tc.tile_pool(name=prefix + "_wv", bufs=1) as wrow:
                for oh in range(2):
                    wvh = []
                    for f in range(NF):
                        wv = wrow.tile([P, 512], F32R, tag="wv", bufs=NF)
                        nc.sync.dma_start(
                            wv[:],
                            w_d[prefix + "_v"][f * P:(f + 1) * P,
                                               oh * 512:(oh + 1) * 512])
                        wvh.append(wv)
                    for tt in range(NT):
                        ps = proj_ps.tile([P, 512], F32, tag="proj")
                        for f in range(NF):
                            nc.tensor.matmul(
                                ps[:], kvsrc_tiles[f][:, tt * P:(tt + 1) * P],
                                wvh[f][:], start=(f == 0), stop=(f == NF - 1))
                        vt = vaug[tt][oh]
                        v3 = vt[:].rearrange("p (h e) -> p h e", e=65)
                        nc.vector.tensor_copy(
                            v3[:, :, 0:DK],
                            ps[:].rearrange("p (h e) -> p h e", e=DK))
                        nc.vector.tensor_copy(
                            v3[:, :, DK:DK + 1],
                            ones_f[:].rearrange("p (h o) -> p h o", o=1))

            if post_proj is not None:
                post_proj()

            # resident additive masks (generic fallback path only)
            cms = None
            if mask_mode == "generic":
                cm_pool = phase_stack.enter_context(
                    tc.tile_pool(name=prefix + "_cm", bufs=1))
                cms = [cm_pool.tile([P, TQ], F32, tag="cm", bufs=NT,
                                    name=f"cm{j}") for j in range(NT)]
                for j in range(NT):
                    nc.scalar.dma_start(cms[j][:], mask_dram[j])

            # attention core, one head pair at a time
            core_pool = phase_stack.enter_context(
                tc.tile_pool(name=prefix + "_core", bufs=1))
            pair_tiles = [core_pool.tile([P, TQ], F32R, tag="pair", bufs=NF,
                                         name=f"pair{i}")
                          for i in range(NF)]
            for hp in range(NF):
                ap_h = [attn_ps.tile([65, TQ], F32, tag="attn", name=f"apsum{hp}_{i}")
                        for i in range(2)]
                for j in range(NT):
                    c0 = _s0(j, mask_mode == "causal") * P
                    for idx in range(2):
                        h = 2 * hp + idx
                        base = idx * DK
                        sps = score_ps.tile([P, TQ], F32, tag="score")
                        nc.tensor.matmul(
                            sps[:, c0:TQ],
                            KT[hp][base:base + DK, j * P:(j + 1) * P],
                            QT[hp][base:base + DK, c0:TQ],
                            start=True, stop=True)
                        probs = core_pool.tile([P, TQ], F32R, tag="probs",
                                               bufs=3)
                        if mask_mode == "generic":
                            ptmp = core_pool.tile([P, TQ], F32, tag="ptmp",
                                                  bufs=2)
                            nc.vector.scalar_tensor_tensor(
                                ptmp[:, c0:TQ], sps[:, c0:TQ], 0.125,
                                cms[j][:, c0:TQ], ALU.mult, ALU.add)
                            nc.scalar.activation(probs[:, c0:TQ],
                                                 ptmp[:, c0:TQ], AF.Exp)
                        else:
                            nc.scalar.activation(probs[:, c0:TQ], sps[:, c0:TQ],
                                                 AF.Exp, scale=0.125)
                        if mask_mode == "causal":
                            # causal fix-ups on 128x128 blocks, in place
                            if j < 4:
                                bq = slice(j * P, (j + 1) * P)
                                nc.vector.tensor_tensor(
                                    probs[:, bq], probs[:, bq], tri[:],
                                    ALU.mult)
                                if j == 3:
                                    nc.vector.tensor_scalar_mul(
                                        probs[:, 2 * P:3 * P],
                                        probs[:, 2 * P:3 * P], zcol[:, 0:1])
                            else:
                                t = j - 4
                                if t >= _s0(j, True):
                                    bq = slice(t * P, (t + 1) * P)
                                    nc.vector.tensor_scalar_mul(
                                        probs[:, bq], probs[:, bq],
                                        pm_t[:, 0:1])
                                if j == 7:
                                    nc.vector.tensor_scalar_mul(
                                        probs[:, 2 * P:3 * P],
                                        probs[:, 2 * P:3 * P], zcol[:, 0:1])
                        nc.tensor.matmul(
                            ap_h[idx][0:65, c0:TQ],
                            vaug[j][h // 8][:, (h % 8) * 65:(h % 8) * 65 + 65],
                            probs[:, c0:TQ],
                            start=(j == 0), stop=(j == NT - 1))
                for idx in range(2):
                    h = 2 * hp + idx
                    r = st_pool.tile([1, TQ], F32, tag="rcp")
                    nc.vector.reciprocal(r[:], ap_h[idx][DK:DK + 1, :])
                    rb = core_pool.tile([DK, TQ], F32, tag="rb")
                    nc.gpsimd.partition_broadcast(rb[:], r[:])
                    tmp = core_pool.tile([DK, TQ], F32, tag="atmp")
                    nc.vector.tensor_tensor(tmp[:], ap_h[idx][0:DK, :], rb[:],
                                            ALU.mult)
                    hfin = core_pool.tile([DK, TQ], F32R, tag="hfin")
                    nc.scalar.activation(hfin[:], tmp[:], AF.Identity,
                                         bias=bv_t[prefix][:, h:h + 1])
                    nc.gpsimd.dma_start(
                        pair_tiles[hp][idx * DK:(idx + 1) * DK, :], hfin[:])

            # O projection + bias + residual
            def o_evict(og, cc, c0, c1, ps):
                nc.vector.scalar_tensor_tensor(
                    out_tiles(og)[:], ps[:], bias_t[prefix + "_o"][:, og:og + 1],
                    res_tiles[og][:, 0:TQ], ALU.add, ALU.add)
            dense_cols(w_d[prefix + "_o"], NF, NF, TQ, pair_tiles,
                       wpool, "w", 2, o_evict)

        def lazy_tiles(st, pname, tag, n, dtype=F32R):
            """Right-side pool whose tiles appear at first use."""
            state = {}

            def get(i):
                if "tiles" not in state:
                    pool = st.enter_context(
                        tc.tile_pool(name=pname, bufs=1, side="right"))
                    state["tiles"] = [
                        pool.tile([P, TQ], dtype, tag=tag, bufs=n,
                                  name=f"{pname}{k}") for k in range(n)]
                return state["tiles"][i]
            return get

        # ================= self attention =================
        sa_st = ExitStack()
        xTr_st = ExitStack()
        xTr_pool = xTr_st.enter_context(
            tc.tile_pool(name="xTr", bufs=1, side="right"))
        xTr = [xTr_pool.tile([P, S], F32R, tag="xTr", bufs=NF, name=f"xTr{i}")
               for i in range(NF)]
        with tc.tile_pool(name="tm_x", bufs=1) as tm_pool:
            load_transposed(x_d, xTr, tm_pool)

        xq_pool = sa_st.enter_context(tc.tile_pool(name="xTq", bufs=1))
        xTq = []

        def sa_post_proj():
            for f in range(NF):
                xt = xq_pool.tile([P, TQ], F32R, tag="xTq", bufs=NF,
                                  name=f"xTq{f}")
                nc.vector.tensor_copy(xt[:], xTr[f][:, 0:TQ])
                xTq.append(xt)
            xTr_st.close()

        sares_st = ExitStack()
        saresT = lazy_tiles(sares_st, "sares", "res", NF)
        attention("sa", xTr, xTr, "causal" if causal else "generic",
                  cmask_d, xTq, saresT, sa_st, post_proj=sa_post_proj)
        sa_st.close()

        # ================= LN1 =================
        ln_pool = top.enter_context(tc.tile_pool(name="ln_pool", bufs=1))
        xn_pool = top.enter_context(tc.tile_pool(name="xn_pool", bufs=NF))
        x1T = [xn_pool.tile([P, TQ], F32R, tag="xn", bufs=NF, name=f"x1T{i}")
               for i in range(NF)]
        layer_norm([saresT(i) for i in range(NF)], x1T, ln_t["g1"], ln_t["b1"])
        sares_st.close()

        # ================= cross attention =================
        ca_st = ExitStack()
        encT_st = ExitStack()
        encT_pool = encT_st.enter_context(
            tc.tile_pool(name="encT", bufs=1, side="right"))
        encT = [encT_pool.tile([P, S], F32R, tag="encT", bufs=NF,
                               name=f"encT{i}")
                for i in range(NF)]
        with tc.tile_pool(name="tm_e", bufs=1) as tm_pool:
            load_transposed(enc_d, encT, tm_pool)

        cares_st = ExitStack()
        caresT = lazy_tiles(cares_st, "cares", "res", NF)
        attention("ca", x1T, encT, "generic" if ca_mask else None,
                  camask_d, x1T, caresT, ca_st, post_proj=encT_st.close)
        ca_st.close()

        x2T = [xn_pool.tile([P, TQ], F32R, tag="xn", bufs=NF, name=f"x2T{i}")
               for i in range(NF)]
        layer_norm([caresT(i) for i in range(NF)], x2T, ln_t["g2"], ln_t["b2"])
        cares_st.close()

        # ================= FFN =================
        ffn_st = ExitStack()
        h1_pool = ffn_st.enter_context(tc.tile_pool(name="h1T", bufs=1))
        h1T = [h1_pool.tile([P, TQ], F32R, tag="h1T", bufs=FF // P,
                            name=f"h1T{i}")
               for i in range(FF // P)]
        wf_pool = ffn_st.enter_context(tc.tile_pool(name="wf", bufs=1))

        def fc1_evict(og, cc, c0, c1, ps):
            nc.scalar.activation(h1T[og][:], ps[:], AF.Relu,
                                 bias=bias_t["fc1"][:, og:og + 1])
        dense_cols(w_d["fc1"], NF, FF // P, TQ, x2T, wf_pool, "w1", 3, fc1_evict)

        ffres_st = ExitStack()
        ffresT = lazy_tiles(ffres_st, "ffres", "res", NF)

        def fc2_evict(og, cc, c0, c1, ps):
            nc.vector.scalar_tensor_tensor(
                ffresT(og)[:], ps[:], bias_t["fc2"][:, og:og + 1],
                x2T[og][:, 0:TQ], ALU.add, ALU.add)
        dense_cols(w_d["fc2"], FF // P, NF, TQ, h1T, wf_pool, "w2", 2,
                   fc2_evict, ogb=1)
        ffn_st.close()

        yT = [xn_pool.tile([P, TQ], F32, tag="xn", bufs=NF, name=f"yT{i}")
              for i in range(NF)]
        layer_norm([ffresT(i) for i in range(NF)], yT, ln_t["g3"], ln_t["b3"],
                   dst_dtype=F32)
        ffres_st.close()

        # ================= transpose out + store =================
        with tc.tile_pool(name="out", bufs=1) as out_pool:
            out_sb = [out_pool.tile([P, D], F32, tag="out", bufs=QS,
                                name=f"osb{i}")
                      for i in range(QS)]
            for og in range(NF):
                for t in range(QS):
                    ps = misc_ps.tile([P, TQ], F32, tag="proj")
                    nc.tensor.transpose(ps[:, 0:P], yT[og][:, t * P:(t + 1) * P],
                                        identity[:])
                    nc.vector.tensor_copy(out_sb[t][:, og * P:(og + 1) * P],
                                          ps[:, 0:P])
            for t in range(QS):
                nc.gpsimd.dma_start(y_d[t * P:(t + 1) * P, :], out_sb[t][:])

    nc.compile()
    return nc


# =====================================================================
# Host side
# =====================================================================

def _perm_for(p):
    mine = np.concatenate(
        [np.arange(t * P, (t + 1) * P) for t in range(p, NT, 2)])
    rest = np.concatenate(
        [np.arange(t * P, (t + 1) * P) for t in range(1 - p, NT, 2)])
    return np.concatenate([mine, rest])


def _build_masks(tgt, perm, causal):
    """Additive masks [NT, P, TQ] in permuted token order for one parity."""
    q_glob = perm[:TQ]
    m = tgt[np.ix_(q_glob, perm)]                  # [TQ, S], 1 = keep
    add = np.where(m.T == 1, 0.0, NEG).astype(np.float32)   # [S, TQ]
    out = np.ascontiguousarray(add.reshape(NT, P, TQ))
    if causal:
        # verify suffix skipping is sound: slots below s0(j) fully masked
        for j in range(NT):
            c0 = _s0(j, True) * P
            if c0 > 0 and not (out[j, :, :c0] == NEG).all():
                return None
    return out


_CACHE = {}

REPLICATED = frozenset(
    ["cvec", "cvec64", "fc1_W", "fc2_W"]
    + [nm + "_W" for nm in
       ["sa_q", "sa_k", "sa_v", "sa_o", "ca_q", "ca_k", "ca_v", "ca_o"]])


def _get_runner(causal, ca_mask):
    key = (causal, ca_mask)
    if key in _CACHE:
        return _CACHE[key]
    import jax
    from jax.sharding import Mesh, PartitionSpec
    from jax.experimental.shard_map import shard_map
    from concourse import bass2jax

    nc = build_program(causal=causal, ca_mask=ca_mask)
    bass2jax.install_neuronx_cc_hook()

    pid_name = (nc.partition_id_tensor.name
                if nc.partition_id_tensor is not None else None)
    in_names, out_names, out_avals, zero_outs = [], [], [], []
    for alloc in nc.m.functions[0].allocations:
        if not isinstance(alloc, mybir.MemoryLocationSet):
            continue
        name = alloc.memorylocations[0].name
        if alloc.kind == "ExternalInput":
            if name != pid_name:
                in_names.append(name)
        elif alloc.kind == "ExternalOutput":
            out_names.append(name)
            shape = tuple(alloc.tensor_shape)
            dtype = mybir.dt.np(alloc.dtype)
            out_avals.append(jax.core.ShapedArray(shape, dtype))
            zero_outs.append(np.zeros(shape, dtype))
    n_params = len(in_names)
    all_in_names = in_names + out_names
    if pid_name is not None:
        all_in_names = all_in_names + [pid_name]

    def _body(*args):
        operands = list(args)
        if pid_name is not None:
            operands.append(bass2jax.partition_id_tensor())
        outs = bass2jax._bass_exec_p.bind(
            *operands,
            out_avals=tuple(out_avals),
            in_names=tuple(all_in_names),
            out_names=tuple(out_names),
            lowering_input_output_aliases=(),
            sim_require_finite=True,
            sim_require_nnan=True,
            nc=nc,
        )
        return tuple(outs)

    devices = jax.devices()[:N_CORES]
    mesh = Mesh(np.asarray(devices), ("core",))
    n_outs = len(out_names)
    in_specs = tuple(
        PartitionSpec() if nm in REPLICATED else PartitionSpec("core")
        for nm in in_names) + (PartitionSpec("core"),) * n_outs
    sharded = jax.jit(
        shard_map(_body, mesh=mesh,
                  in_specs=in_specs,
                  out_specs=(PartitionSpec("core"),) * n_outs,
                  check_rep=False),
        donate_argnums=tuple(range(n_params, n_params + n_outs)),
        keep_unused=True,
    )

    runner = (sharded, in_names, out_names, zero_outs, mesh, in_specs)
    _CACHE[key] = runner
    return runner


def _prepare_inputs(x, enc_output, src_mask, tgt_mask, params):
    x = np.asarray(x, np.float32)
    enc = np.asarray(enc_output, np.float32)
    tgt = np.asarray(tgt_mask).reshape(S, S)
    src = np.asarray(src_mask).reshape(-1)
    ca_mask = not (src == 1).all()

    perms = [_perm_for(0), _perm_for(1)]
    causal = bool(np.array_equal(tgt, np.tril(np.ones((S, S), tgt.dtype))))
    masks = None
    if not causal:
        masks = [_build_masks(tgt, perms[p], False) for p in (0, 1)]

    # batched constant vectors (identical on every core)
    def fv(v):
        return np.asarray(v, np.float32).reshape(-1, P).T
    src_map = {
        "sa_q": params["sa_q_b"], "sa_k": params["sa_k_b"],
        "sa_o": params["sa_o_b"], "ca_q": params["ca_q_b"],
        "ca_k": params["ca_k_b"], "ca_o": params["ca_o_b"],
        "fc2": params["fc2_b"], "fc1": params["fc1_b"],
        "g1": params["ln1_g"], "b1": params["ln1_b"],
        "g2": params["ln2_g"], "b2": params["ln2_b"],
        "g3": params["ln3_g"], "b3": params["ln3_b"],
    }
    cvec = np.zeros((P, CVEC_COLS), np.float32)
    for nm, off, n in CVEC_ORDER:
        cvec[:, off:off + n] = fv(src_map[nm])
    cvec64 = np.concatenate(
        [np.asarray(params["sa_v_b"], np.float32).reshape(H, DK).T,
         np.asarray(params["ca_v_b"], np.float32).reshape(H, DK).T], axis=1)

    shared = {"cvec": cvec, "cvec64": np.ascontiguousarray(cvec64)}
    for nm in ["sa_q", "sa_k", "sa_v", "sa_o", "ca_q", "ca_k", "ca_v", "ca_o"]:
        shared[nm + "_W"] = np.asarray(params[nm + "_W"], np.float32)
    shared["fc1_W"] = np.asarray(params["fc1_W"], np.float32)
    shared["fc2_W"] = np.asarray(params["fc2_W"], np.float32)

    in_maps = []
    for c in range(N_CORES):
        b, p = c // 2, c % 2
        m = dict(shared)
        m["x"] = np.ascontiguousarray(x[b][perms[p]])
        m["enc"] = np.ascontiguousarray(enc[b])
        if causal:
            m["pm"] = np.full((P, 1), float(p), np.float32)
        else:
            m["cmask"] = masks[p]
        if ca_mask:
            sm = np.where(src == 1, 0.0, NEG).astype(np.float32)
            cam = np.broadcast_to(sm[:, None], (S, TQ)).reshape(NT, P, TQ)
            m["camask"] = np.ascontiguousarray(cam)
        in_maps.append(m)
    return in_maps, causal, ca_mask, perms


def _global_inputs(in_maps, in_names):
    return [in_maps[0][nm] if nm in REPLICATED
            else np.concatenate([m[nm] for m in in_maps], axis=0)
            for nm in in_names]


def kernel(x, enc_output, src_mask, tgt_mask, params):
    in_maps, causal, ca_mask, perms = _prepare_inputs(
        x, enc_output, src_mask, tgt_mask, params)
    sharded, in_names, out_names, zero_outs, mesh, in_specs = _get_runner(
        causal, ca_mask)

    concat_in = _global_inputs(in_maps, in_names)
    concat_zeros = [np.zeros((N_CORES * z.shape[0], *z.shape[1:]), z.dtype)
                    for z in zero_outs]
    out_arrs = sharded(*concat_in, *concat_zeros)
    yi = out_names.index("y")
    ys = np.asarray(out_arrs[yi]).reshape(N_CORES, TQ, D)

    out = np.empty((N_CORES // 2, S, D), np.float32)
    for c in range(N_CORES):
        b, p = c // 2, c % 2
        out[b][perms[p][:TQ]] = ys[c]
    return out
